# revision 13
# baseline (speedup 1.0000x reference)
"""Trainium2 Bass kernel for a top-2-of-8 MoE layer (attention-pooled gating).

Strategy
--------
The reference computes every expert densely and combines with weights ``g``
that have exactly K=2 nonzeros per batch (softmax -> top-k mask -> renorm).
So the mathematically identical computation is: route each batch to its top-2
experts and compute only those 64 (batch, expert) pairs.

Host side (cheap): fp32 gating mirroring the reference op-for-op, top-2
selection, renormalized weights.  The 64 pairs are then packed into an
SPMD-uniform schedule: every core gets the same *run pattern* (e.g. [4,3,1])
where each run is a maximal set of same-expert pairs.  A run loads its
expert's weights once and streams all its batches through them.  An exact
cover solver picks the largest feasible pattern for the observed routing
(fallback chain down to [1]*8 == one expert-load per pair).

Device side (the heavy 1.37e11 FLOPs): per run, two matmul layers in
transposed layout, contraction on the partition axis:
    hT[h,s]  = gelu(sum_d w1[d,h] * xT[d,s] + b1[h])
    eoT[o,s] = gelu(sum_h w2[h,o] * hT[h,s] + b2[o])
The batch loop is innermost so consecutive matmuls share the stationary
weight tile; a post-Tile IR pass then deletes the redundant LDWEIGHTS the
compiler would otherwise emit per matmul (hardware keeps the PE weight
state across matmuls).  PE dtype float16 (fp32 PSUM accumulation), outputs
written fp16 and combined on host: out[b] = sum_k g_k * eoT_k^T.
"""

import os

import numpy as np

import jax

jax.config.update(
    "jax_compilation_cache_dir", os.path.expanduser("~/.jax_bass_cache")
)
jax.config.update("jax_persistent_cache_min_compile_time_secs", 0)
jax.config.update("jax_persistent_cache_min_entry_size_bytes", 0)

import concourse.bacc as bacc
import concourse.mybir as mybir
import concourse.tile as tile
from concourse.tile_rust import add_dep_helper
from concourse.bass_utils import run_bass_kernel_spmd

B, S, D = 32, 512, 512
E, H, O, K = 8, 2048, 512, 2
NCORES = 8
NB = (B * K) // NCORES  # 8 (batch, expert) pair slots per core

# PE dtype: float16 -- same 1 cyc/row speed as bf16, but a 10-bit mantissa
# (~4x less rounding error).  All values here are small (|x|<6, |w|<0.2,
# |z|<3), so fp16 range is not a concern.
MM_DT = mybir.dt.float16
NP_MM_DT = np.float16
F32 = mybir.dt.float32

DT_TILES = D // 128   # 4 k-tiles for layer 1
HT_TILES = H // 128   # 16 h-tiles
OT_TILES = O // 128   # 4 o-tiles

DEDUP_LDW = True
FORCE_ORDER = False
L1_CHUNK = 2   # batches per layer-1 weight-sharing group (PSUM banks used)
L2_CHUNK = 2   # batches per layer-2 weight-sharing group
PS1_BUFS = 4
PS2_BUFS = 4

# Run patterns to try, best first.  All parts <= 4 so a run's accumulation
# chains fit in 4 PSUM banks (4 for layer 1 + 4 for layer 2 = all 8).
_PATTERNS = [
    [4, 3, 1], [4, 2, 2], [3, 3, 2], [4, 2, 1, 1], [3, 3, 1, 1],
    [3, 2, 2, 1], [2, 2, 2, 2], [2, 2, 2, 1, 1], [4, 1, 1, 1, 1],
    [3, 2, 1, 1, 1], [2, 2, 1, 1, 1, 1], [2, 1, 1, 1, 1, 1, 1],
    [1] * 8,
]

_nc_cache: dict = {}


def _ap_key(arg):
    ks = []
    for attr in ("memref", "memsetref", "offset", "dtype"):
        ks.append(str(getattr(arg, attr, None)))
    ks.append(str(getattr(arg, "ap", None)))
    return tuple(ks)


def _dedup_ldweights(nc):
    """Remove Ldweights whose weights AP equals the PE's current weight
    state (hardware keeps the stationary operand across matmuls).  Only
    sync-free duplicates are dropped; dependency names are remapped to the
    kept instruction."""
    n_removed = 0
    PE = mybir.EngineType.PE
    for blk in nc.main_func.blocks:
        state = None  # (ap key, kept inst name)
        renames = {}
        keep = []
        pending_waits = []  # waits from dropped Ldweights -> next PE inst
        for inst in blk.instructions:
            if inst.engine != PE:
                keep.append(inst)
                continue
            op = inst.opcode
            if op == "Ldweights":
                si = inst.sync_info
                no_update = si is None or not si.on_update
                key = _ap_key(inst.ins[0])
                if state is not None and state[0] == key and no_update:
                    renames[inst.name] = state[1]
                    if si is not None and si.on_wait:
                        pending_waits.extend(si.on_wait)
                    n_removed += 1
                    continue
                state = (key, inst.name)
            elif op == "Matmult":
                if getattr(inst, "ldweights", False):
                    state = None
            elif op in ("Drain", "EventSemaphore", "Nop", "ISA",
                        "RegisterMove"):
                pass
            else:
                state = None
            if pending_waits:
                si = inst.sync_info
                if si is None:
                    inst.sync_info = mybir.SyncInfo(
                        on_wait=list(pending_waits), on_update=[]
                    )
                else:
                    inst.sync_info = mybir.SyncInfo(
                        on_wait=list(pending_waits) + list(si.on_wait),
                        on_update=list(si.on_update),
                    )
                pending_waits = []
            keep.append(inst)
        assert not pending_waits, "dropped Ldweights waits had no landing inst"
        if renames:
            blk.instructions[:] = keep
            for inst in keep:
                inst.remap_dependency_names(renames)
    return n_removed


def _build(pattern, repeat: int = 1):
    """Build + compile the per-core SPMD program for a run pattern.

    repeat > 1 wraps the whole body in a hardware loop -- used only for
    timing (the body is idempotent)."""
    pattern = tuple(pattern)
    key = (pattern, repeat)
    if key in _nc_cache:
        return _nc_cache[key]
    nruns = len(pattern)

    nc = bacc.Bacc(
        "TRN2", target_bir_lowering=False, debug=False, num_devices=NCORES
    )
    xT_d = nc.dram_tensor("xT", [NB, D, S], MM_DT, kind="ExternalInput")
    w1_d = nc.dram_tensor("w1g", [nruns, D, H], MM_DT, kind="ExternalInput")
    w2_d = nc.dram_tensor("w2g", [nruns, H, O], MM_DT, kind="ExternalInput")
    b1_d = nc.dram_tensor(
        "b1g", [nruns, 128, HT_TILES], F32, kind="ExternalInput"
    )
    b2_d = nc.dram_tensor(
        "b2g", [nruns, 128, OT_TILES], F32, kind="ExternalInput"
    )
    out_d = nc.dram_tensor("outT", [NB, O, S], MM_DT, kind="ExternalOutput")

    with tile.TileContext(nc) as tc:
        with (
            tc.tile_pool(name="xp", bufs=NB) as xp,
            tc.tile_pool(name="w1p", bufs=2) as w1p,
            tc.tile_pool(name="w2p", bufs=2) as w2p,
            tc.tile_pool(name="bp", bufs=2) as bp,
            tc.tile_pool(name="hp", bufs=4) as hp,
            tc.tile_pool(name="op", bufs=5) as op,
            tc.tile_pool(name="ps1", bufs=PS1_BUFS, space="PSUM") as ps1,
            tc.tile_pool(name="ps2", bufs=PS2_BUFS, space="PSUM") as ps2,
        ):
            prev_mm = [None]

            def mm(out, lhsT, rhs, **kw):
                inst = nc.tensor.matmul(out, lhsT, rhs, **kw)
                inst = getattr(inst, "ins", inst)
                if FORCE_ORDER and prev_mm[0] is not None:
                    add_dep_helper(inst, prev_mm[0], sync=False,
                                   reason="force PE stream order")
                prev_mm[0] = inst

            def run_inputs(r, slots):
                # All input DMAs go on the HWDGE (SP) queue, issued for
                # every run up-front so later runs' weights prefetch under
                # earlier runs' compute.  Order within a run: the first L1
                # matmuls need w1's first H-chunk AND the x tiles; w2 is
                # not needed until layer 2.
                w1t = w1p.tile([128, DT_TILES, H], MM_DT, tag="w1", name="w1t")
                w1src = w1_d[r].rearrange("(d q) h -> q d h", q=128)
                hs = slice(0, H // 4)
                nc.sync.dma_start(w1t[:, :, hs], w1src[:, :, hs])
                xts = []
                for slot in slots:
                    xt = xp.tile([128, DT_TILES, S], MM_DT, tag="xt", name="xt")
                    nc.sync.dma_start(
                        xt[:], xT_d[slot].rearrange("(d q) s -> q d s", q=128)
                    )
                    xts.append(xt)
                b1t = bp.tile([128, HT_TILES], F32, tag="b1", name="b1t")
                nc.sync.dma_start(b1t[:], b1_d[r])
                b2t = bp.tile([128, OT_TILES], F32, tag="b2", name="b2t")
                nc.sync.dma_start(b2t[:], b2_d[r])
                for c in range(1, 4):
                    hs = slice(c * (H // 4), (c + 1) * (H // 4))
                    nc.sync.dma_start(w1t[:, :, hs], w1src[:, :, hs])
                w2t = w2p.tile([128, HT_TILES, O], MM_DT, tag="w2", name="w2t")
                nc.sync.dma_start(
                    w2t[:], w2_d[r].rearrange("(t q) o -> q t o", q=128)
                )
                return w1t, w2t, b1t, b2t, xts

            def run_compute(r, slots, tiles):
                sr = len(slots)
                w1t, w2t, b1t, b2t, xts = tiles

                hts = [
                    hp.tile([128, HT_TILES, S], MM_DT, tag="ht", name="ht")
                    for _ in range(sr)
                ]
                for t in range(HT_TILES):
                    wcol = slice(t * 128, (t + 1) * 128)
                    for j0 in range(0, sr, L1_CHUNK):
                        js = range(j0, min(j0 + L1_CHUNK, sr))
                        pss = {j: ps1.tile([128, S], F32, tag="ps1", name="ps1t")
                               for j in js}
                        for d in range(DT_TILES):
                            for j in js:
                                mm(
                                    pss[j][:],
                                    w1t[:, d, wcol],
                                    xts[j][:, d, :],
                                    start=(d == 0),
                                    stop=(d == DT_TILES - 1),
                                )
                        for j in js:
                            nc.scalar.activation(
                                hts[j][:, t, :],
                                pss[j][:],
                                mybir.ActivationFunctionType.Gelu,
                                bias=b1t[:, t : t + 1],
                            )

                ots = [
                    op.tile([128, OT_TILES, S], MM_DT, tag="ot", name="ot")
                    for _ in range(sr)
                ]
                for o in range(OT_TILES):
                    wcol = slice(o * 128, (o + 1) * 128)
                    for j0 in range(0, sr, L2_CHUNK):
                        js = range(j0, min(j0 + L2_CHUNK, sr))
                        pss = {j: ps2.tile([128, S], F32, tag="ps2", name="ps2t")
                               for j in js}
                        for t in range(HT_TILES):
                            for j in js:
                                mm(
                                    pss[j][:],
                                    w2t[:, t, wcol],
                                    hts[j][:, t, :],
                                    start=(t == 0),
                                    stop=(t == HT_TILES - 1),
                                )
                        for j in js:
                            nc.scalar.activation(
                                ots[j][:, o, :],
                                pss[j][:],
                                mybir.ActivationFunctionType.Gelu,
                                bias=b2t[:, o : o + 1],
                            )
                # outputs on the SWDGE (gpsimd) queue: a dma_start for an
                # output waits on its ACTs, and on the shared HWDGE queue
                # that wait would head-of-line-block the next run's (and
                # next iteration's) input DMAs.
                for j, slot in enumerate(slots):
                    nc.gpsimd.dma_start(
                        out_d[slot].rearrange("(t q) s -> q t s", q=128),
                        ots[j][:],
                    )

            def body():
                slot_ranges = []
                slot = 0
                for sr in pattern:
                    slot_ranges.append(list(range(slot, slot + sr)))
                    slot += sr
                tiles = [
                    run_inputs(r, slots)
                    for r, slots in enumerate(slot_ranges)
                ]
                for r, slots in enumerate(slot_ranges):
                    run_compute(r, slots, tiles[r])

            if repeat == 1:
                body()
            else:
                with tc.For_i(0, repeat, 1, staggered_reset=True):
                    body()

    if DEDUP_LDW:
        _dedup_ldweights(nc)
    nc.compile()
    _nc_cache[key] = nc
    return nc


def _gating(x, attn_w, attn_b, gate_w, gate_b):
    """fp32 gating, op-for-op with the reference. Returns (idx [B,K], gn [B,K])."""
    f32 = np.float32
    x = x.astype(f32, copy=False)
    scores = x @ attn_w.astype(f32) + attn_b.astype(f32)          # [B,S,1]
    scores = scores - scores.max(axis=1, keepdims=True)
    e = np.exp(scores)
    aw = e / e.sum(axis=1, keepdims=True)
    pooled = (x * aw).sum(axis=1)                                  # [B,D]
    logits = pooled @ gate_w.astype(f32) + gate_b.astype(f32)      # [B,E]
    logits = logits - logits.max(axis=-1, keepdims=True)
    ge = np.exp(logits)
    gates = ge / ge.sum(axis=-1, keepdims=True)
    # top-k with lower-index tie-break, like lax.top_k
    idx = np.argsort(-gates, axis=-1, kind="stable")[:, :K]        # [B,K]
    gg = np.take_along_axis(gates, idx, axis=-1)
    gn = gg / (gg.sum(axis=-1, keepdims=True) + f32(1e-9))
    return idx, gn


def _decompose(counts, supply):
    """Exactly decompose each expert count into parts from `supply`
    (size -> available number).  Returns {expert: [part,...]} or None."""
    items = sorted(counts.items(), key=lambda kv: -kv[1])
    sizes = sorted(supply, reverse=True)

    def combos(target, avail, start):
        """All multisets of parts summing to target within avail (pure)."""
        if target == 0:
            return [{}]
        out = []
        for i in range(start, len(sizes)):
            s = sizes[i]
            if s <= target and avail[s] > 0:
                avail[s] -= 1
                for rest in combos(target - s, avail, i):
                    c = dict(rest)
                    c[s] = c.get(s, 0) + 1
                    out.append(c)
                avail[s] += 1
        return out

    def bt(i, avail):
        if i == len(items):
            return {} if all(v == 0 for v in avail.values()) else None
        e, cnt = items[i]
        for combo in combos(cnt, avail, 0):
            for s, n in combo.items():
                avail[s] -= n
            r = bt(i + 1, avail)
            if r is not None:
                r[e] = [s for s, n in combo.items() for _ in range(n)]
                return r
            for s, n in combo.items():
                avail[s] += n
        return None

    return bt(0, dict(supply))


def _solve_schedule(idx, gn):
    """Pack the 64 (batch, expert, gate) pairs into an SPMD-uniform
    schedule: every core gets runs sized per the chosen pattern, each run
    single-expert.  Returns (pattern, cores) with cores[c] = [(expert,
    [(batch, gate), ...]), ...] in pattern order."""
    from collections import Counter, defaultdict

    per_e = defaultdict(list)
    for b in range(B):
        for k in range(K):
            per_e[int(idx[b, k])].append((b, float(gn[b, k])))
    counts = {e: len(v) for e, v in per_e.items()}

    for pat in _PATTERNS:
        supply = {s: NCORES * c for s, c in Counter(pat).items()}
        decomp = _decompose(counts, supply)
        if decomp is None:
            continue
        runs_by_size = defaultdict(list)
        for e in sorted(decomp):
            for p in sorted(decomp[e], reverse=True):
                runs_by_size[p].append(e)
        cursor = {e: 0 for e in per_e}
        size_pos = {s: 0 for s in supply}
        cores = []
        for c in range(NCORES):
            runs = []
            for s in pat:
                e = runs_by_size[s][size_pos[s]]
                size_pos[s] += 1
                lst = per_e[e][cursor[e] : cursor[e] + s]
                cursor[e] += s
                runs.append((e, lst))
            cores.append(runs)
        return tuple(pat), cores
    raise RuntimeError("no feasible pattern (unreachable: [1]*8 is always ok)")


def _prepare(x, attn_w, attn_b, gate_w, gate_b, w1, b1, w2, b2):
    """Gating + scheduling + per-core input packing."""
    x = np.asarray(x)
    idx, gn = _gating(
        x, np.asarray(attn_w), np.asarray(attn_b), np.asarray(gate_w),
        np.asarray(gate_b),
    )
    pattern, cores = _solve_schedule(idx, gn)

    w1_c = np.ascontiguousarray(np.asarray(w1)).astype(NP_MM_DT)   # [E,D,H]
    w2_c = np.ascontiguousarray(np.asarray(w2)).astype(NP_MM_DT)   # [E,H,O]
    xT_c = np.ascontiguousarray(x.transpose(0, 2, 1)).astype(NP_MM_DT)
    b1_t = np.ascontiguousarray(
        np.asarray(b1).reshape(E, HT_TILES, 128).transpose(0, 2, 1)
    ).astype(np.float32)                                           # [E,128,16]
    b2_t = np.ascontiguousarray(
        np.asarray(b2).reshape(E, OT_TILES, 128).transpose(0, 2, 1)
    ).astype(np.float32)                                           # [E,128,4]

    in_maps = []
    for c in range(NCORES):
        es = [e for (e, lst) in cores[c]]
        bs = [b for (e, lst) in cores[c] for (b, g) in lst]
        in_maps.append(
            {
                "xT": xT_c[bs],
                "w1g": w1_c[es],
                "w2g": w2_c[es],
                "b1g": b1_t[es],
                "b2g": b2_t[es],
            }
        )
    return pattern, cores, in_maps


def kernel(
    x, attn_w, attn_b, gate_w, gate_b, w1, b1, w2, b2
) -> np.ndarray:
    pattern, cores, in_maps = _prepare(
        x, attn_w, attn_b, gate_w, gate_b, w1, b1, w2, b2
    )
    nc = _build(pattern, repeat=1)
    br = run_bass_kernel_spmd(nc, in_maps, list(range(NCORES)))

    out = np.zeros((B, S, O), np.float32)
    for c in range(NCORES):
        eoT = br.results[c]["outT"].astype(np.float32)             # [NB,O,S]
        slot = 0
        for e, lst in cores[c]:
            for b, g in lst:
                out[b] += np.float32(g) * eoT[slot].T
                slot += 1
    return out


# revision 14
# speedup vs baseline: 1.3751x; 1.3751x over previous
"""Trainium2 Bass kernel for a top-2-of-8 MoE layer (attention-pooled gating).

Strategy
--------
The reference computes every expert densely and combines with weights ``g``
that have exactly K=2 nonzeros per batch (softmax -> top-k mask -> renorm).
So the mathematically identical computation is: route each batch to its top-2
experts and compute only those 64 (batch, expert) pairs.

Host side (cheap, O(B*S*D)): attention-pool gating in fp32 mirroring the
reference op-for-op, top-2 selection, renormalized weights.  The 64 pairs are
sorted by expert and dealt 8-per-core across the 8 NeuronCores (perfect
compute balance regardless of expert skew).  Inputs are pre-gathered and
pre-transposed per pair so the device kernel is fully static.

Device side (the heavy 1.37e11 FLOPs): per pair, two matmul layers in
transposed layout, contraction on the partition axis:
    hT[h,s]  = gelu(sum_d w1[d,h] * xT[d,s] + b1[h])     (16 h-tiles x 4 k-mm)
    eoT[o,s] = gelu(sum_h w2[h,o] * hT[h,s] + b2[o])     (4 o-tiles x 16 k-mm)
Weights/acts run through the PE in float16 (fp32 PSUM accumulation); biases
and outputs are fp32.  Host combines: out[b] = (g0*eoT0 + g1*eoT1)^T.
"""

import os

import numpy as np

import jax

jax.config.update(
    "jax_compilation_cache_dir", os.path.expanduser("~/.jax_bass_cache")
)
jax.config.update("jax_persistent_cache_min_compile_time_secs", 0)
jax.config.update("jax_persistent_cache_min_entry_size_bytes", 0)

import concourse.bacc as bacc
import concourse.mybir as mybir
import concourse.tile as tile
from concourse.bass_utils import run_bass_kernel_spmd

B, S, D = 32, 512, 512
E, H, O, K = 8, 2048, 512, 2
NCORES = 8
PAIRS = (B * K) // NCORES  # 8 (batch, expert) pairs per core

# PE dtype: float16 -- same 1 cyc/row + FWL speed as bf16, but a 10-bit
# mantissa (~4x less rounding error).  All values here are small (|x|<6,
# |w|<0.2, |z|<3), so fp16 range is not a concern.
MM_DT = mybir.dt.float16
NP_MM_DT = np.float16
F32 = mybir.dt.float32

DT_TILES = D // 128   # 4 k-tiles for layer 1
HT_TILES = H // 128   # 16 h-tiles
OT_TILES = O // 128   # 4 o-tiles

_nc_cache: dict = {}


def _build(repeat: int = 1):
    """Build + compile the per-core SPMD program (identical on all cores).

    repeat > 1 wraps the whole body in a hardware loop -- used only for
    timing (the body is idempotent)."""
    key = repeat
    if key in _nc_cache:
        return _nc_cache[key]

    nc = bacc.Bacc(
        "TRN2", target_bir_lowering=False, debug=False, num_devices=NCORES
    )
    xT_d = nc.dram_tensor("xT", [PAIRS, D, S], MM_DT, kind="ExternalInput")
    w1_d = nc.dram_tensor("w1g", [PAIRS, D, H], MM_DT, kind="ExternalInput")
    w2_d = nc.dram_tensor("w2g", [PAIRS, H, O], MM_DT, kind="ExternalInput")
    b1_d = nc.dram_tensor("b1g", [PAIRS, 128, HT_TILES], F32, kind="ExternalInput")
    b2_d = nc.dram_tensor("b2g", [PAIRS, 128, OT_TILES], F32, kind="ExternalInput")
    out_d = nc.dram_tensor("outT", [PAIRS, O, S], F32, kind="ExternalOutput")

    with tile.TileContext(nc) as tc:
        with (
            tc.tile_pool(name="xp", bufs=3) as xp,
            tc.tile_pool(name="w1p", bufs=3) as w1p,
            tc.tile_pool(name="w2p", bufs=3) as w2p,
            tc.tile_pool(name="bp", bufs=3) as bp,
            tc.tile_pool(name="hp", bufs=2) as hp,
            tc.tile_pool(name="op", bufs=3) as op,
            tc.tile_pool(name="ps1", bufs=4, space="PSUM") as ps1,
            tc.tile_pool(name="ps2", bufs=4, space="PSUM") as ps2,
        ):

            def pair_body(p):
                xt = xp.tile([128, DT_TILES, S], MM_DT)
                nc.sync.dma_start(
                    xt[:], xT_d[p].rearrange("(t q) s -> q t s", q=128)
                )
                w1t = w1p.tile([128, DT_TILES, H], MM_DT)
                nc.sync.dma_start(
                    w1t[:], w1_d[p].rearrange("(t q) h -> q t h", q=128)
                )
                w2t = w2p.tile([128, HT_TILES, O], MM_DT)
                nc.sync.dma_start(
                    w2t[:], w2_d[p].rearrange("(t q) o -> q t o", q=128)
                )
                b1t = bp.tile([128, HT_TILES], F32, tag="b1")
                nc.sync.dma_start(b1t[:], b1_d[p])
                b2t = bp.tile([128, OT_TILES], F32, tag="b2")
                nc.sync.dma_start(b2t[:], b2_d[p])

                ht = hp.tile([128, HT_TILES, S], MM_DT)
                for t in range(HT_TILES):
                    ps = ps1.tile([128, S], F32)
                    for d in range(DT_TILES):
                        nc.tensor.matmul(
                            ps[:],
                            w1t[:, d, t * 128 : (t + 1) * 128],
                            xt[:, d, :],
                            start=(d == 0),
                            stop=(d == DT_TILES - 1),
                        )
                    nc.scalar.activation(
                        ht[:, t, :],
                        ps[:],
                        mybir.ActivationFunctionType.Gelu,
                        bias=b1t[:, t : t + 1],
                    )

                ot = op.tile([128, OT_TILES, S], F32)
                for o in range(OT_TILES):
                    ps = ps2.tile([128, S], F32)
                    for t in range(HT_TILES):
                        nc.tensor.matmul(
                            ps[:],
                            w2t[:, t, o * 128 : (o + 1) * 128],
                            ht[:, t, :],
                            start=(t == 0),
                            stop=(t == HT_TILES - 1),
                        )
                    nc.scalar.activation(
                        ot[:, o, :],
                        ps[:],
                        mybir.ActivationFunctionType.Gelu,
                        bias=b2t[:, o : o + 1],
                    )
                nc.sync.dma_start(
                    out_d[p].rearrange("(t q) s -> q t s", q=128), ot[:]
                )

            if repeat == 1:
                for p in range(PAIRS):
                    pair_body(p)
            else:
                with tc.For_i(0, repeat, 1, staggered_reset=True):
                    for p in range(PAIRS):
                        pair_body(p)

    nc.compile()
    _nc_cache[key] = nc
    return nc


def _gating(x, attn_w, attn_b, gate_w, gate_b):
    """fp32 gating, op-for-op with the reference. Returns (idx [B,K], gn [B,K])."""
    f32 = np.float32
    x = x.astype(f32, copy=False)
    scores = x @ attn_w.astype(f32) + attn_b.astype(f32)          # [B,S,1]
    scores = scores - scores.max(axis=1, keepdims=True)
    e = np.exp(scores)
    aw = e / e.sum(axis=1, keepdims=True)
    pooled = (x * aw).sum(axis=1)                                  # [B,D]
    logits = pooled @ gate_w.astype(f32) + gate_b.astype(f32)      # [B,E]
    logits = logits - logits.max(axis=-1, keepdims=True)
    ge = np.exp(logits)
    gates = ge / ge.sum(axis=-1, keepdims=True)
    # top-k with lower-index tie-break, like lax.top_k
    idx = np.argsort(-gates, axis=-1, kind="stable")[:, :K]        # [B,K]
    gg = np.take_along_axis(gates, idx, axis=-1)
    gn = gg / (gg.sum(axis=-1, keepdims=True) + f32(1e-9))
    return idx, gn


def _schedule(idx, gn):
    """64 (b, e, g) pairs -> NCORES lists of PAIRS, grouped by expert."""
    pairs = [
        (int(idx[b, k]), b, float(gn[b, k])) for b in range(B) for k in range(K)
    ]
    pairs.sort()  # by expert, then batch: same-expert pairs land adjacently
    return [pairs[c * PAIRS : (c + 1) * PAIRS] for c in range(NCORES)]


def kernel(
    x, attn_w, attn_b, gate_w, gate_b, w1, b1, w2, b2
) -> np.ndarray:
    x = np.asarray(x)
    idx, gn = _gating(
        x, np.asarray(attn_w), np.asarray(attn_b), np.asarray(gate_w),
        np.asarray(gate_b),
    )
    sched = _schedule(idx, gn)

    w1 = np.asarray(w1)
    w2 = np.asarray(w2)
    b1 = np.asarray(b1)
    b2 = np.asarray(b2)
    w1_c = np.ascontiguousarray(w1).astype(NP_MM_DT)               # [E,D,H]
    w2_c = np.ascontiguousarray(w2).astype(NP_MM_DT)               # [E,H,O]
    xT_c = np.ascontiguousarray(x.transpose(0, 2, 1)).astype(NP_MM_DT)  # [B,D,S]
    b1_t = np.ascontiguousarray(
        b1.reshape(E, HT_TILES, 128).transpose(0, 2, 1)
    ).astype(np.float32)                                           # [E,128,16]
    b2_t = np.ascontiguousarray(
        b2.reshape(E, OT_TILES, 128).transpose(0, 2, 1)
    ).astype(np.float32)                                           # [E,128,4]

    in_maps = []
    for c in range(NCORES):
        es = [p[0] for p in sched[c]]
        bs = [p[1] for p in sched[c]]
        in_maps.append(
            {
                "xT": xT_c[bs],
                "w1g": w1_c[es],
                "w2g": w2_c[es],
                "b1g": b1_t[es],
                "b2g": b2_t[es],
            }
        )

    nc = _build(repeat=1)
    br = run_bass_kernel_spmd(nc, in_maps, list(range(NCORES)))

    out = np.zeros((B, S, O), np.float32)
    for c in range(NCORES):
        eoT = br.results[c]["outT"]                                # [PAIRS,O,S]
        for p, (e, b, g) in enumerate(sched[c]):
            out[b] += np.float32(g) * eoT[p].T
    return out


# revision 16
# speedup vs baseline: 1.5135x; 1.1006x over previous
"""Trainium2 Bass kernel for a top-2-of-8 MoE layer (attention-pooled gating).

Strategy
--------
The reference computes every expert densely and combines with weights ``g``
that have exactly K=2 nonzeros per batch (softmax -> top-k mask -> renorm).
So the mathematically identical computation is: route each batch to its top-2
experts and compute only those 64 (batch, expert) pairs.

Host side (cheap, O(B*S*D)): attention-pool gating in fp32 mirroring the
reference op-for-op, top-2 selection, renormalized weights.  The 64 pairs are
sorted by expert and dealt 8-per-core across the 8 NeuronCores (perfect
compute balance regardless of expert skew).  Inputs are pre-gathered and
pre-transposed per pair so the device kernel is fully static.

Device side (the heavy 1.37e11 FLOPs): per pair, two matmul layers in
transposed layout, contraction on the partition axis:
    hT[h,s]  = gelu(sum_d w1[d,h] * xT[d,s] + b1[h])     (16 h-tiles x 4 k-mm)
    eoT[o,s] = gelu(sum_h w2[h,o] * hT[h,s] + b2[o])     (4 o-tiles x 16 k-mm)
Weights/acts run through the PE in float16 (fp32 PSUM accumulation); biases
are fp32.  Outputs are written fp16 (halves the output DMA; ~5e-4 rel err
contribution) and the w1 weight DMA is issued in four H-chunks so the first
h-tiles' matmuls can start before the full 2MB lands.  Host combines in
fp32: out[b] = (g0*eoT0 + g1*eoT1)^T.
"""

import os

import numpy as np

import jax

jax.config.update(
    "jax_compilation_cache_dir", os.path.expanduser("~/.jax_bass_cache")
)
jax.config.update("jax_persistent_cache_min_compile_time_secs", 0)
jax.config.update("jax_persistent_cache_min_entry_size_bytes", 0)

import concourse.bacc as bacc
import concourse.mybir as mybir
import concourse.tile as tile
from concourse.bass_utils import run_bass_kernel_spmd

B, S, D = 32, 512, 512
E, H, O, K = 8, 2048, 512, 2
NCORES = 8
PAIRS = (B * K) // NCORES  # 8 (batch, expert) pairs per core

# PE dtype: float16 -- same 1 cyc/row + FWL speed as bf16, but a 10-bit
# mantissa (~4x less rounding error).  All values here are small (|x|<6,
# |w|<0.2, |z|<3), so fp16 range is not a concern.
MM_DT = mybir.dt.float16
NP_MM_DT = np.float16
F32 = mybir.dt.float32

DT_TILES = D // 128   # 4 k-tiles for layer 1
HT_TILES = H // 128   # 16 h-tiles
OT_TILES = O // 128   # 4 o-tiles

_nc_cache: dict = {}


def _build(repeat: int = 1):
    """Build + compile the per-core SPMD program (identical on all cores).

    repeat > 1 wraps the whole body in a hardware loop -- used only for
    timing (the body is idempotent)."""
    key = repeat
    if key in _nc_cache:
        return _nc_cache[key]

    nc = bacc.Bacc(
        "TRN2", target_bir_lowering=False, debug=False, num_devices=NCORES
    )
    xT_d = nc.dram_tensor("xT", [PAIRS, D, S], MM_DT, kind="ExternalInput")
    w1_d = nc.dram_tensor("w1g", [PAIRS, D, H], MM_DT, kind="ExternalInput")
    w2_d = nc.dram_tensor("w2g", [PAIRS, H, O], MM_DT, kind="ExternalInput")
    b1_d = nc.dram_tensor("b1g", [PAIRS, 128, HT_TILES], F32, kind="ExternalInput")
    b2_d = nc.dram_tensor("b2g", [PAIRS, 128, OT_TILES], F32, kind="ExternalInput")
    out_d = nc.dram_tensor("outT", [PAIRS, O, S], MM_DT, kind="ExternalOutput")

    with tile.TileContext(nc) as tc:
        with (
            tc.tile_pool(name="xp", bufs=3) as xp,
            tc.tile_pool(name="w1p", bufs=3) as w1p,
            tc.tile_pool(name="w2p", bufs=3) as w2p,
            tc.tile_pool(name="bp", bufs=3) as bp,
            tc.tile_pool(name="hp", bufs=2) as hp,
            tc.tile_pool(name="op", bufs=3) as op,
            tc.tile_pool(name="ps1", bufs=4, space="PSUM") as ps1,
            tc.tile_pool(name="ps2", bufs=4, space="PSUM") as ps2,
        ):

            def pair_body(p):
                xt = xp.tile([128, DT_TILES, S], MM_DT)
                nc.sync.dma_start(
                    xt[:], xT_d[p].rearrange("(t q) s -> q t s", q=128)
                )
                w1t = w1p.tile([128, DT_TILES, H], MM_DT)
                w1src = w1_d[p].rearrange("(t q) h -> q t h", q=128)
                for hc in range(4):
                    hs = slice(hc * (H // 4), (hc + 1) * (H // 4))
                    nc.sync.dma_start(w1t[:, :, hs], w1src[:, :, hs])
                w2t = w2p.tile([128, HT_TILES, O], MM_DT)
                nc.sync.dma_start(
                    w2t[:], w2_d[p].rearrange("(t q) o -> q t o", q=128)
                )
                b1t = bp.tile([128, HT_TILES], F32, tag="b1")
                nc.sync.dma_start(b1t[:], b1_d[p])
                b2t = bp.tile([128, OT_TILES], F32, tag="b2")
                nc.sync.dma_start(b2t[:], b2_d[p])

                ht = hp.tile([128, HT_TILES, S], MM_DT)
                for t in range(HT_TILES):
                    ps = ps1.tile([128, S], F32)
                    for d in range(DT_TILES):
                        nc.tensor.matmul(
                            ps[:],
                            w1t[:, d, t * 128 : (t + 1) * 128],
                            xt[:, d, :],
                            start=(d == 0),
                            stop=(d == DT_TILES - 1),
                        )
                    nc.scalar.activation(
                        ht[:, t, :],
                        ps[:],
                        mybir.ActivationFunctionType.Gelu,
                        bias=b1t[:, t : t + 1],
                    )

                ot = op.tile([128, OT_TILES, S], MM_DT)
                for o in range(OT_TILES):
                    ps = ps2.tile([128, S], F32)
                    for t in range(HT_TILES):
                        nc.tensor.matmul(
                            ps[:],
                            w2t[:, t, o * 128 : (o + 1) * 128],
                            ht[:, t, :],
                            start=(t == 0),
                            stop=(t == HT_TILES - 1),
                        )
                    nc.scalar.activation(
                        ot[:, o, :],
                        ps[:],
                        mybir.ActivationFunctionType.Gelu,
                        bias=b2t[:, o : o + 1],
                    )
                nc.sync.dma_start(
                    out_d[p].rearrange("(t q) s -> q t s", q=128), ot[:]
                )

            if repeat == 1:
                for p in range(PAIRS):
                    pair_body(p)
            else:
                with tc.For_i(0, repeat, 1, staggered_reset=True):
                    for p in range(PAIRS):
                        pair_body(p)

    nc.compile()
    _nc_cache[key] = nc
    return nc


def _gating(x, attn_w, attn_b, gate_w, gate_b):
    """fp32 gating, op-for-op with the reference. Returns (idx [B,K], gn [B,K])."""
    f32 = np.float32
    x = x.astype(f32, copy=False)
    scores = x @ attn_w.astype(f32) + attn_b.astype(f32)          # [B,S,1]
    scores = scores - scores.max(axis=1, keepdims=True)
    e = np.exp(scores)
    aw = e / e.sum(axis=1, keepdims=True)
    pooled = (x * aw).sum(axis=1)                                  # [B,D]
    logits = pooled @ gate_w.astype(f32) + gate_b.astype(f32)      # [B,E]
    logits = logits - logits.max(axis=-1, keepdims=True)
    ge = np.exp(logits)
    gates = ge / ge.sum(axis=-1, keepdims=True)
    # top-k with lower-index tie-break, like lax.top_k
    idx = np.argsort(-gates, axis=-1, kind="stable")[:, :K]        # [B,K]
    gg = np.take_along_axis(gates, idx, axis=-1)
    gn = gg / (gg.sum(axis=-1, keepdims=True) + f32(1e-9))
    return idx, gn


def _schedule(idx, gn):
    """64 (b, e, g) pairs -> NCORES lists of PAIRS, grouped by expert."""
    pairs = [
        (int(idx[b, k]), b, float(gn[b, k])) for b in range(B) for k in range(K)
    ]
    pairs.sort()  # by expert, then batch: same-expert pairs land adjacently
    return [pairs[c * PAIRS : (c + 1) * PAIRS] for c in range(NCORES)]


def kernel(
    x, attn_w, attn_b, gate_w, gate_b, w1, b1, w2, b2
) -> np.ndarray:
    x = np.asarray(x)
    idx, gn = _gating(
        x, np.asarray(attn_w), np.asarray(attn_b), np.asarray(gate_w),
        np.asarray(gate_b),
    )
    sched = _schedule(idx, gn)

    w1 = np.asarray(w1)
    w2 = np.asarray(w2)
    b1 = np.asarray(b1)
    b2 = np.asarray(b2)
    w1_c = np.ascontiguousarray(w1).astype(NP_MM_DT)               # [E,D,H]
    w2_c = np.ascontiguousarray(w2).astype(NP_MM_DT)               # [E,H,O]
    xT_c = np.ascontiguousarray(x.transpose(0, 2, 1)).astype(NP_MM_DT)  # [B,D,S]
    b1_t = np.ascontiguousarray(
        b1.reshape(E, HT_TILES, 128).transpose(0, 2, 1)
    ).astype(np.float32)                                           # [E,128,16]
    b2_t = np.ascontiguousarray(
        b2.reshape(E, OT_TILES, 128).transpose(0, 2, 1)
    ).astype(np.float32)                                           # [E,128,4]

    in_maps = []
    for c in range(NCORES):
        es = [p[0] for p in sched[c]]
        bs = [p[1] for p in sched[c]]
        in_maps.append(
            {
                "xT": xT_c[bs],
                "w1g": w1_c[es],
                "w2g": w2_c[es],
                "b1g": b1_t[es],
                "b2g": b2_t[es],
            }
        )

    nc = _build(repeat=1)
    br = run_bass_kernel_spmd(nc, in_maps, list(range(NCORES)))

    out = np.zeros((B, S, O), np.float32)
    for c in range(NCORES):
        eoT = br.results[c]["outT"].astype(np.float32)             # [PAIRS,O,S]
        for p, (e, b, g) in enumerate(sched[c]):
            out[b] += np.float32(g) * eoT[p].T
    return out
